# revision 1
# baseline (speedup 1.0000x reference)
"""GCN encoder (2-layer) on 8 Trainium2 NeuronCores.

Math (per layer, matching the reference):
    out[d] = dis[d] * sum_{e: dst_e=d} dis[src_e] * h[src_e]  + b
with h = x @ W, dis = deg^-1/2 over src-with-self-loops. dis factors are
folded host-side: xT is pre-scaled by dis (layer-1 operand), layer-1's
output scaling uses dis^2 (post relu identity: dis*relu(z) = relu(dis*z)),
layer 2 applies dis at the end.

This environment executes roughly one engine instruction per ~55us with no
cross-engine overlap, so the design minimizes instruction count:
  - edges per dst-window (128 dsts) are gathered in [rank, slot] order so
    token k*128+p is the k-th in-edge of window-slot p; one wide
    tensor_reduce over the rank axis aggregates a whole window.
  - dma_gather with single_packet=False allows ~8192 indices/instruction
    (single_packet=True hangs above ~1024).
  - rank padding points at injected all-zero rows: every core ships 6251
    rows (row 6250 zeroed), so zero rows exist in both the lo ([0,32768))
    and hi ([32768,50008)) gather bases of the int16-index split.
Sharding: nodes row-sharded 6250/core, edges partitioned by dst core,
weights replicated, AllGather between layers.
"""
import os
import numpy as np

N, E = 50000, 1600000
FIN, FHID, FOUT = 256, 128, 64
NCORES = 8
NPC = N // NCORES          # 6250
NPC2 = NPC + 1             # 6251 rows shipped per core (last = zeros)
NFULL = NCORES * NPC2      # 50008
NW = (NPC + 127) // 128    # 49 windows
NPAD = NW * 128            # 6272
HALF = 32768               # int16 gather base split
ZLO = 6250                 # zero row inside lo base (core 0 pad row)
ZHI = 5 * NPC2 + NPC - HALF  # core 5 pad row, hi-base-local index
MAXRANKS = 64              # ranks per gather instruction (8192 idxs)

_CACHE = {}
LAST_RESULTS = None


def _host_prep(x, edge_index, W1, b1, W2, b2):
    x = np.asarray(x, dtype=np.float32)
    ei = np.asarray(edge_index)
    W1 = np.asarray(W1, dtype=np.float32)
    W2 = np.asarray(W2, dtype=np.float32)
    b1 = np.asarray(b1, dtype=np.float32)
    b2 = np.asarray(b2, dtype=np.float32)

    loops = np.arange(N, dtype=np.int64)
    src = np.concatenate([ei[0].astype(np.int64), loops])
    dst = np.concatenate([ei[1].astype(np.int64), loops])

    deg = np.bincount(src, minlength=N).astype(np.float32)
    dis = np.power(deg, np.float32(-0.5), dtype=np.float32)
    dis[deg == 0] = 0.0

    # padded gather row of each source node
    r_all = (src // NPC) * NPC2 + (src % NPC)
    s_all = (r_all >= HALF).astype(np.int64)  # 0 = lo stream, 1 = hi

    core = dst // NPC
    order = np.argsort(dst, kind="stable")
    r_s, dst_s, s_s = r_all[order], dst[order], s_all[order]
    cb = np.searchsorted(dst_s, np.arange(NCORES + 1) * NPC)

    # per-core rank assignment within (dst, stream)
    percore = []
    KLO = np.zeros((NCORES, NW), np.int64)
    KHI = np.zeros((NCORES, NW), np.int64)
    for c in range(NCORES):
        sl = slice(cb[c], cb[c + 1])
        r_c = r_s[sl]
        d_c = dst_s[sl] - c * NPC
        s_c = s_s[sl]
        key = d_c * 2 + s_c
        o2 = np.argsort(key, kind="stable")
        key_o = key[o2]
        first = np.searchsorted(key_o, key_o, side="left")
        rank = np.arange(len(key_o)) - first
        d_o, s_o, r_o = d_c[o2], s_c[o2], r_c[o2]
        w_o, p_o = d_o // 128, d_o % 128
        np.maximum.at(KLO[c], w_o[s_o == 0], rank[s_o == 0] + 1)
        np.maximum.at(KHI[c], w_o[s_o == 1], rank[s_o == 1] + 1)
        percore.append((w_o, p_o, s_o, rank, r_o))

    KLOm = KLO.max(axis=0)  # [NW]
    KHIm = KHI.max(axis=0)
    # pad window pairs (2w, 2w+1) to equal total ranks so one 4D-AP
    # tensor_reduce can aggregate both windows at once
    Kt = KLOm + KHIm
    for i in range(0, NW - 1, 2):
        kp = max(Kt[i], Kt[i + 1])
        KHIm[i] += kp - Kt[i]
        KHIm[i + 1] += kp - Kt[i + 1]
    K = KLOm + KHIm
    # flat token-position offsets: window w = [lo ranks][hi ranks]
    woff = np.zeros(NW + 1, np.int64)
    woff[1:] = np.cumsum(K) * 128
    total_tok = int(woff[-1])

    in_maps = []
    for c in range(NCORES):
        w_o, p_o, s_o, rank, r_o = percore[c]
        gidx = np.empty(total_tok, np.int16)
        for w in range(NW):
            gidx[woff[w]:woff[w] + KLOm[w] * 128] = ZLO
            gidx[woff[w] + KLOm[w] * 128:woff[w + 1]] = ZHI
        pos = woff[w_o] + (rank + np.where(s_o == 1, KLOm[w_o], 0)) * 128 + p_o
        gidx[pos] = np.where(s_o == 1, r_o - HALF, r_o).astype(np.int16)
        gidx_t = np.tile(gidx.reshape(-1, 16).T, (8, 1))  # [128, total_tok//16]

        dis_l = dis[c * NPC:(c + 1) * NPC]
        dis_pad = np.zeros(NPAD, np.float32)
        dis_pad[:NPC] = dis_l
        dis_col = np.ascontiguousarray(dis_pad.reshape(NW, 128).T)  # [128, NW]
        dis2_col = dis_col * dis_col
        # Bstt[p, w*128+f] = dis[w*128+p] * b1[f]
        Bstt = (dis_col.T[:, :, None] * b1[None, None, :]).transpose(1, 0, 2)
        Bstt = np.ascontiguousarray(Bstt.reshape(128, NW * FHID))

        xT = np.zeros((FIN, NPAD), np.float32)
        xT[:, :NPC] = (x[c * NPC:(c + 1) * NPC] * dis_l[:, None]).T

        in_maps.append({
            "gidx": np.ascontiguousarray(gidx_t),
            "xT": xT,
            "W1": W1, "W2": W2,
            "dis2c": dis2_col, "disc": dis_col,
            "Bstt": Bstt,
            "b2b": np.tile(b2, (128, 1)),
            "ident": np.eye(128, dtype=np.float32),
        })
    return in_maps, (KLOm, KHIm, bool(not b1.any()))


def _build(Kinfo):
    import concourse.bacc as bacc
    import concourse.mybir as mybir
    import concourse.tile as tile

    KLOm, KHIm, B1ZERO = Kinfo
    K = KLOm + KHIm
    maxK = max(int(K[i]) * (1 if i + 1 >= NW else 2)
               for i in range(0, NW, 2))
    total_tok = int(K.sum()) * 128

    PHASES = os.environ.get("GCN_PHASES", "full")
    REPEAT = int(os.environ.get("GCN_REPEAT", "1"))

    dt = mybir.dt
    ALU = mybir.AluOpType

    nc = bacc.Bacc("TRN2", target_bir_lowering=False, debug=False,
                   num_devices=NCORES)

    gidx_d = nc.dram_tensor("gidx", [128, total_tok // 16], dt.int16, kind="ExternalInput")
    xT_d = nc.dram_tensor("xT", [FIN, NPAD], dt.float32, kind="ExternalInput")
    W1_d = nc.dram_tensor("W1", [FIN, FHID], dt.float32, kind="ExternalInput")
    W2_d = nc.dram_tensor("W2", [FHID, FOUT], dt.float32, kind="ExternalInput")
    dis2_d = nc.dram_tensor("dis2c", [128, NW], dt.float32, kind="ExternalInput")
    dis_d = nc.dram_tensor("disc", [128, NW], dt.float32, kind="ExternalInput")
    Bstt_d = nc.dram_tensor("Bstt", [128, NW * FHID], dt.float32, kind="ExternalInput")
    b2b_d = nc.dram_tensor("b2b", [128, FOUT], dt.float32, kind="ExternalInput")
    ident_d = nc.dram_tensor("ident", [128, 128], dt.float32, kind="ExternalInput")
    out_d = nc.dram_tensor("out", [NPC, FOUT], dt.float32, kind="ExternalOutput")

    t1_local = nc.dram_tensor("t1_local", [NPC2, FHID], dt.float32)
    t1_full = nc.dram_tensor("t1_full", [NFULL, FHID], dt.float32, addr_space="Shared")
    t2_local = nc.dram_tensor("t2_local", [NPC2, FOUT], dt.float32)
    t2_full = nc.dram_tensor("t2_full", [NFULL, FOUT], dt.float32, addr_space="Shared")

    with tile.TileContext(nc) as tc:
        with (
            tc.tile_pool(name="consts", bufs=1) as cp,
            tc.tile_pool(name="work", bufs=1) as wp,
            tc.tile_pool(name="psum", bufs=1, space="PSUM") as pp,
        ):
            ident_t = cp.tile([128, 128], dt.float32, tag="ident")
            nc.sync.dma_start(ident_t[:], ident_d[:, :])
            w1_t = cp.tile([128, 2, FHID], dt.float32, tag="w1")
            nc.sync.dma_start(w1_t[:, 0, :], W1_d[0:128, :])
            nc.sync.dma_start(w1_t[:, 1, :], W1_d[128:256, :])
            w2_t = cp.tile([FHID, FOUT], dt.float32, tag="w2")
            nc.sync.dma_start(w2_t[:], W2_d[:, :])
            dis2_t = cp.tile([128, NW], dt.float32, tag="dis2")
            nc.sync.dma_start(dis2_t[:], dis2_d[:, :])
            dis_t = cp.tile([128, NW], dt.float32, tag="dis")
            nc.sync.dma_start(dis_t[:], dis_d[:, :])
            if not B1ZERO:
                Bstt_t = cp.tile([128, NW * FHID], dt.float32, tag="Bstt")
                nc.sync.dma_start(Bstt_t[:], Bstt_d[:, :])
            b2b_t = cp.tile([128, FOUT], dt.float32, tag="b2b")
            nc.sync.dma_start(b2b_t[:], b2b_d[:, :])
            gidx_t = cp.tile([128, total_tok // 16], dt.int16, tag="gidx")
            nc.sync.dma_start(gidx_t[:], gidx_d[:, :])
            zrow = cp.tile([128, FHID], dt.float32, tag="zrow")
            nc.vector.memset(zrow[:], 0.0)

            # one shared gpsimd register per distinct gather count: avoids a
            # RegisterMove instruction (~55us here) per dma_gather
            counts = set()
            for w in range(NW):
                for nk in (int(KLOm[w]), int(KHIm[w])):
                    for k0 in range(0, nk, MAXRANKS):
                        counts.add(min(MAXRANKS, nk - k0) * 128)
            nidx_regs = {cnt: nc.gpsimd.to_reg(cnt) for cnt in sorted(counts)}

            for _rep in range(REPEAT):
                # ---- phase B: t1_local = (dis*x) @ W1 ----
                with tc.tile_pool(name="phaseB", bufs=1) as pb:
                    xT_t = pb.tile([128, 2, NPAD], dt.float32, tag="xT")
                    nc.sync.dma_start(xT_t[:, 0, :], xT_d[0:128, :])
                    nc.sync.dma_start(xT_t[:, 1, :], xT_d[128:256, :])
                    evB = pb.tile([128, 8, FHID], dt.float32, tag="evB")
                    psB = pp.tile([128, 8, FHID], dt.float32, tag="pB")
                    for w in range(NW):
                        sl = psB[:, w % 8, :]
                        for kc in range(2):
                            nc.tensor.matmul(
                                sl, xT_t[:, kc, w * 128:w * 128 + 128],
                                w1_t[:, kc, :], start=(kc == 0), stop=(kc == 1))
                        if w % 8 == 7:
                            nc.vector.tensor_copy(evB[:], psB[:])
                        if w == 48:
                            nc.vector.tensor_copy(evB[:, 0, :], sl)
                        if w % 8 == 7:
                            nc.sync.dma_start(
                                t1_local[(w - 7) * 128:(w + 1) * 128, :]
                                .rearrange("(a p) f -> p a f", p=128),
                                evB[:])
                    # window 48 (106 rows)
                    nc.sync.dma_start(t1_local[48 * 128:NPC, :],
                                      evB[0:106, 0, :])
                    nc.sync.dma_start(t1_local[NPC:NPC2, :], zrow[0:1, :])

                nc.gpsimd.collective_compute(
                    "AllGather", mybir.AluOpType.bypass,
                    replica_groups=[list(range(NCORES))],
                    ins=[t1_local[:, :]], outs=[t1_full[:, :]],
                )

                if PHASES == "B":
                    ot = wp.tile([128, FOUT], dt.float32, tag="o")
                    nc.vector.memset(ot[:], 0.0)
                    for w in range(NW):
                        rows = min(128, NPC - w * 128)
                        nc.sync.dma_start(out_d[w * 128:w * 128 + rows, :],
                                          ot[0:rows, :])
                    continue

                def gather_window(tok, w, src_full, feat, woff_w, dk=0):
                    """Emit gathers for window w into tok at rank offset dk."""
                    klo, khi = int(KLOm[w]), int(KHIm[w])
                    base_lo = src_full[0:HALF, :]
                    base_hi = src_full[HALF:NFULL, :]
                    segs = [(0, klo, base_lo), (klo, khi, base_hi)]
                    for seg0, nk, base in segs:
                        for k0 in range(0, nk, MAXRANKS):
                            kn = min(MAXRANKS, nk - k0)
                            c0 = (woff_w + (seg0 + k0) * 128) // 16
                            d0 = dk + seg0 + k0
                            nc.gpsimd.dma_gather(
                                tok[:, d0:d0 + kn, :], base,
                                gidx_t[:, c0:c0 + kn * 8],
                                num_idxs=kn * 128,
                                num_idxs_reg=nidx_regs[kn * 128],
                                elem_size=feat, single_packet=False)

                # ---- L1 pass 1: gather + reduce + scale into o1s_all ----
                with tc.tile_pool(name="L1", bufs=1) as l1:
                    tok = l1.tile([128, maxK, FHID], dt.float32, tag="tok1")
                    red = l1.tile([128, 2, FHID], dt.float32, tag="red")
                    o1s_all = l1.tile([128, NW, FHID], dt.float32, tag="o1sa")
                    o1T = l1.tile([128, 4, FHID], dt.float32, tag="o1T")
                    ev1 = l1.tile([128, 8, FOUT], dt.float32, tag="ev1")
                    pT = pp.tile([128, 4, 512], dt.float32, tag="pT")  # slice per bank
                    p2 = pp.tile([128, 8, FOUT], dt.float32, tag="p2")
                    woff_w = 0
                    for w0 in range(0, NW, 2):
                        pair = [w0] if w0 + 1 >= NW else [w0, w0 + 1]
                        kp = int(K[w0])
                        for j, w in enumerate(pair):
                            gather_window(tok, w, t1_full, FHID,
                                          woff_w, j * kp)
                            woff_w += int(K[w]) * 128
                        nc.vector.tensor_reduce(
                            red[:, 0:len(pair), :],
                            tok[:, 0:len(pair) * kp, :]
                            .rearrange("p (b k) f -> p b f k", b=len(pair)),
                            mybir.AxisListType.X, ALU.add)
                        for j, w in enumerate(pair):
                            # o1s = relu(dis^2*red + dis*b1)
                            if B1ZERO:
                                nc.vector.tensor_scalar(
                                    o1s_all[:, w, :], red[:, j, :],
                                    dis2_t[:, w:w + 1],
                                    0.0, ALU.mult, ALU.max)
                            else:
                                nc.vector.scalar_tensor_tensor(
                                    o1s_all[:, w, :], red[:, j, :],
                                    dis2_t[:, w:w + 1],
                                    Bstt_t[:, w * 128:(w + 1) * 128],
                                    ALU.mult, ALU.add)
                                nc.vector.tensor_scalar(
                                    o1s_all[:, w, :], o1s_all[:, w, :], 0.0,
                                    None, ALU.max)
                    # ---- L1 pass 2: transpose + @W2, batched ----
                    for w in range(NW):
                        nc.tensor.transpose(pT[:, w % 4, 0:FHID],
                                            o1s_all[:, w, :], ident_t[:])
                        if w % 4 == 3:
                            nc.vector.tensor_copy(o1T[:], pT[:, :, 0:FHID])
                        if w == 48:
                            nc.vector.tensor_copy(o1T[:, 0, :], pT[:, 0, 0:FHID])
                        if w % 4 == 3 or w == 48:
                            for w2 in range(w - (3 if w % 4 == 3 else 0), w + 1):
                                nc.tensor.matmul(p2[:, w2 % 8, :],
                                                 o1T[:, w2 % 4, :], w2_t[:],
                                                 start=True, stop=True)
                        if w % 8 == 7:
                            nc.vector.tensor_copy(ev1[:], p2[:])
                        if w == 48:
                            nc.vector.tensor_copy(ev1[:, 0, :], p2[:, 0, :])
                        if w % 8 == 7:
                            nc.sync.dma_start(
                                t2_local[(w - 7) * 128:(w + 1) * 128, :]
                                .rearrange("(a p) f -> p a f", p=128),
                                ev1[:])
                    nc.sync.dma_start(t2_local[48 * 128:NPC, :],
                                      ev1[0:106, 0, :])
                    nc.sync.dma_start(t2_local[NPC:NPC2, :], zrow[0:1, 0:FOUT])

                if PHASES == "B1":
                    ot = wp.tile([128, FOUT], dt.float32, tag="o")
                    nc.vector.memset(ot[:], 0.0)
                    for w in range(NW):
                        rows = min(128, NPC - w * 128)
                        nc.sync.dma_start(out_d[w * 128:w * 128 + rows, :],
                                          ot[0:rows, :])
                    continue

                nc.gpsimd.collective_compute(
                    "AllGather", mybir.AluOpType.bypass,
                    replica_groups=[list(range(NCORES))],
                    ins=[t2_local[:, :]], outs=[t2_full[:, :]],
                )

                # ---- L2 windows ----
                with tc.tile_pool(name="L2", bufs=1) as l2:
                    tok2 = l2.tile([128, maxK, FOUT], dt.float32, tag="tok2")
                    red2 = l2.tile([128, 2, FOUT], dt.float32, tag="red2")
                    ev2 = l2.tile([128, 8, FOUT], dt.float32, tag="ev2")
                    woff_w = 0
                    for w0 in range(0, NW, 2):
                        pair = [w0] if w0 + 1 >= NW else [w0, w0 + 1]
                        kp = int(K[w0])
                        for j, w in enumerate(pair):
                            gather_window(tok2, w, t2_full, FOUT,
                                          woff_w, j * kp)
                            woff_w += int(K[w]) * 128
                        nc.vector.tensor_reduce(
                            red2[:, 0:len(pair), :],
                            tok2[:, 0:len(pair) * kp, :]
                            .rearrange("p (b k) f -> p b f k", b=len(pair)),
                            mybir.AxisListType.X, ALU.add)
                        for j, w in enumerate(pair):
                            nc.vector.scalar_tensor_tensor(
                                ev2[:, w % 8, :], red2[:, j, :],
                                dis_t[:, w:w + 1],
                                b2b_t[:], ALU.mult, ALU.add)
                        w = pair[-1]
                        if w % 8 == 7:
                            nc.sync.dma_start(
                                out_d[(w - 7) * 128:(w + 1) * 128, :]
                                .rearrange("(a p) f -> p a f", p=128),
                                ev2[:])
                    nc.sync.dma_start(out_d[48 * 128:NPC, :], ev2[0:106, 0, :])

    nc.compile()
    return nc


def kernel(x, edge_index, W1, b1, W2, b2):
    global LAST_RESULTS
    from concourse.bass_utils import run_bass_kernel_spmd

    in_maps, Kinfo = _host_prep(x, edge_index, W1, b1, W2, b2)
    key = (Kinfo[0].tobytes(), Kinfo[1].tobytes(), Kinfo[2])
    if key not in _CACHE:
        _CACHE[key] = _build(Kinfo)
    nc = _CACHE[key]

    res = run_bass_kernel_spmd(nc, in_maps, list(range(NCORES)))
    LAST_RESULTS = res
    return np.concatenate([res.results[c]["out"] for c in range(NCORES)], axis=0)

